# revision 50
# baseline (speedup 1.0000x reference)
"""Trainium2 Bass kernel for nn_Node2Pair_bias (LayerNorm -> dual projection ->
pair outer-product -> head-mix linear).

Reference computation (B=2, L=512, D=256, DH=32, H=16, K=2, P=128):
    x   = LayerNorm(node) * gamma + beta, masked        [B, L, D]
    left  = (x @ W_left + b_left)                       [B, L, DH] -> [B,L,H,K]
    right = (x @ W_right + b_right)/sqrt(DH)            [B, L, DH] -> [B,L,H,K]
    out[b,i,j,h] = sum_k left[b,i,h,k]*right[b,j,h,k]
    out[b,i,j,p] = sum_h out[b,i,j,h]*W_out[h,p] + b_out[p]   [B, L, L, P]

Mathematical restructuring (c = (h,k) combined channel, 0..31):
    out[b,i,j,p] = sum_c right[b,j,c] * (left[b,i,c] * W2[c,p]) + b_out[p]
with W2[c,p] = W_out[c//2, p].  For each i, M_i[c,p] = left[b,i,c]*W2[c,p] is
built on a vector-class engine; 4 i's pack side by side into an rhs of
[32, 512], and the K=32 contraction uses only one 32-row group of the PE
array — so 4 consecutive i-blocks (il=0..3) are row-packed via
tile_position=(32*il, 0) and run CONCURRENTLY on disjoint row groups:
  lhsT = rightT_quad[32il:32il+32, j-chunk]   (right values, 4 replicas)
  rhs  = mp_quad[32il:32il+32, (i4, p)=512]
  -> psum_il[j=128, (i4, p)=512]
The partition-replication of rightT/leftT across the 4 row groups comes free
by tiling the projection-weight COLUMNS 4x on the host.  PSUM is drained to
fp16 staging (DVE/ACT) and DMA'd out; the host adds b_out and converts
fp16 -> f32 while un-sharding (the 2e-2 rel-err budget is ~40x the fp16
rounding error).

LayerNorm gamma/beta and both projection biases are folded into the
projection weights (exact algebra): rows = [gamma[:,None]*W; (beta@W) paired
with a mask row; b paired with a ones row].

Sharding: the i axis of L is split across the 8 cores (sequence-parallel);
each core holds its [B, 64] slice of `left` inputs plus the full `right` side
and writes a [B, 64, L, P] output shard.  No cross-device communication.
"""

import sys

sys.path.insert(0, "/opt/trn_rl_repo")

import numpy as np

import concourse.bass as bass  # noqa: F401
import concourse.mybir as mybir
import concourse.tile as tile
from concourse import bacc
from concourse.bass_utils import run_bass_kernel_spmd
from concourse.masks import make_identity

F32 = mybir.dt.float32
F16 = mybir.dt.float16

B, L, D = 2, 512, 256
DH, H, PAIR = 32, 16, 128
NCORES = 8
LSH = L // NCORES          # 64 i's per core per batch
LN_EPS = 1e-5

_COMPILED = None  # (nc, input_names)


def _build_program():
    nc = bacc.Bacc("TRN2", target_bir_lowering=False, debug=False,
                   num_devices=NCORES)

    # ---------------- DRAM parameters ----------------
    def din(name, shape, dt=F32):
        return nc.dram_tensor(name, list(shape), dt, kind="ExternalInput").ap()

    node_full = din("node_full", (B * L, D))        # all rows, (b,l) major
    node_shard = din("node_shard", (B * LSH, D))    # this core's i rows, (b,i)
    mask_col_full = din("mask_col_full", (128, B * L // 128))  # [:, t] = tile t
    mask_col_shard = din("mask_col_shard", (128, 1))
    m2_full = din("m2_full", (B * 2, L), F16)       # per b: [mask row; ones]
    m2_shard = din("m2_shard", (2, B * LSH), F16)   # [mask row; ones]
    # columns tiled 4x (col 32*r + dh = W[:, dh]) so projections emit the
    # 4-replica partition layout row-packing needs
    w_left_e = din("w_left_e", (D + 2, 4 * DH), F16)   # [gamma*W; beta@W; b_l]
    w_right_e = din("w_right_e", (D + 2, 4 * DH), F16)  # scaled by 1/sqrt(DH)
    w2 = din("w2", (4 * DH, 4 * PAIR), F16)  # quad rows, free dim tiled 4x

    # Output layout: [b, jcp, sg, j, jh, i16, p] fp16 — each 1 MiB staging
    # buffer lands as one fully contiguous partition-major stream (8 KiB per
    # partition).  Host un-permutes and upcasts while assembling the output.
    out = nc.dram_tensor("out", [B, 2, 4, 128, 2, 16, PAIR], F16,
                         kind="ExternalOutput").ap()

    NT_FULL = B * L // 128   # 8 LayerNorm tiles for the full sequence

    with tile.TileContext(nc) as tc:
        with (
            tc.tile_pool(name="singles", bufs=1) as singles,
            tc.tile_pool(name="xpool", bufs=9) as xpool,
            tc.tile_pool(name="stats", bufs=4) as stats,
            tc.tile_pool(name="persist", bufs=1) as persist,
            tc.tile_pool(name="mp", bufs=4) as mp_pool,
            tc.tile_pool(name="stag", bufs=6) as stag_pool,
            tc.tile_pool(name="ps_tp", bufs=1, space="PSUM") as ps_tp,
            tc.tile_pool(name="ps_proj", bufs=1, space="PSUM") as ps_proj,
            tc.tile_pool(name="ps_big", bufs=3, space="PSUM") as ps_big,
        ):
            # ---------------- constants ----------------
            ident = singles.tile([128, 128], F32, tag="ident")
            make_identity(nc, ident)
            eps_t = singles.tile([128, 1], F32, tag="eps")
            nc.vector.memset(eps_t, LN_EPS)

            # hot-path loads on sync (HWDGE) in dependency-critical order
            # (small weights first so nothing queues behind the node tiles);
            # b=1 tiles + right-side constants via gpsimd (SWDGE) in parallel
            xs = xpool.tile([128, D], F32, tag="x", name="xs")
            nc.sync.dma_start(out=xs, in_=node_shard[:, :])
            mcs_sb = singles.tile([128, 1], F32, tag="mcs")
            nc.sync.dma_start(out=mcs_sb, in_=mask_col_shard[:, :])
            wl_sb = [singles.tile([128, 4 * DH], F16, tag=f"wl{dc}",
                                  name=f"wl{dc}") for dc in range(2)]
            for dc in range(2):
                nc.sync.dma_start(out=wl_sb[dc],
                                  in_=w_left_e[dc * 128:(dc + 1) * 128, :])
            wl_mo = singles.tile([2, 4 * DH], F16, tag="wlmo")
            nc.sync.dma_start(out=wl_mo, in_=w_left_e[D:D + 2, :])
            w2_sb = singles.tile([4 * DH, 4 * PAIR], F16, tag="w2")
            nc.sync.dma_start(out=w2_sb, in_=w2[:, :])
            # shard mask+ones rhs rows [2, B*LSH]
            m2s = singles.tile([2, B * LSH], F16, tag="m2s")
            nc.sync.dma_start(out=m2s, in_=m2_shard[:, :])
            # b=0 node tiles + mask on the scalar HWDGE queue — the ACT
            # sequencer is idle this early, so these land in parallel with
            # the sync queue's weight loads instead of queuing behind them
            mcf_sb = singles.tile([128, NT_FULL], F32, tag="mcf")
            nc.scalar.dma_start(out=mcf_sb, in_=mask_col_full[:, :])
            xf_tiles = [None] * NT_FULL
            for t in range(4):
                xf = xpool.tile([128, D], F32, tag="x", name=f"xf{t}")
                nc.scalar.dma_start(out=xf,
                                    in_=node_full[t * 128:(t + 1) * 128, :])
                xf_tiles[t] = xf

            wr_sb = [singles.tile([128, 4 * DH], F16, tag=f"wr{dc}",
                                  name=f"wr{dc}") for dc in range(2)]
            for dc in range(2):
                nc.gpsimd.dma_start(out=wr_sb[dc],
                                    in_=w_right_e[dc * 128:(dc + 1) * 128, :])
            wr_mo = singles.tile([2, 4 * DH], F16, tag="wrmo")
            nc.gpsimd.dma_start(out=wr_mo, in_=w_right_e[D:D + 2, :])
            m2f = [singles.tile([2, L], F16, tag=f"m2f{b}", name=f"m2f{b}")
                   for b in range(B)]
            for b in range(B):
                nc.gpsimd.dma_start(out=m2f[b],
                                    in_=m2_full[2 * b:2 * b + 2, :])
            for t in range(4, NT_FULL):
                xf = xpool.tile([128, D], F32, tag="x", name=f"xf{t}")
                nc.gpsimd.dma_start(out=xf,
                                    in_=node_full[t * 128:(t + 1) * 128, :])
                xf_tiles[t] = xf

            # ---------------- LayerNorm helper ----------------
            def layernorm_masked(x_t, mask_col_ap):
                """x_t [128, D] in place -> (x - mu) * rsqrt(var+eps) * mask."""
                st = stats.tile([128, 6], F32, tag="st")
                nc.vector.bn_stats(out=st, in_=x_t)
                mv = stats.tile([128, 2], F32, tag="mv")
                nc.vector.bn_aggr(out=mv, in_=st)
                sd = stats.tile([128, 1], F32, tag="sd")
                nc.scalar.activation(out=sd, in_=mv[:, 1:2],
                                     func=mybir.ActivationFunctionType.Sqrt,
                                     bias=eps_t, scale=1.0)
                rs = stats.tile([128, 1], F32, tag="rs")
                nc.vector.reciprocal(out=rs, in_=sd)
                rsm = stats.tile([128, 1], F32, tag="rsm")
                nc.vector.tensor_mul(out=rsm, in0=rs, in1=mask_col_ap)
                nc.vector.tensor_scalar(out=x_t, in0=x_t,
                                        scalar1=mv[:, 0:1], scalar2=rsm,
                                        op0=mybir.AluOpType.subtract,
                                        op1=mybir.AluOpType.mult)

            # ---------------- shard path: leftT_r [128, B*LSH] ---------------
            layernorm_masked(xs, mcs_sb[:, 0:1])

            xsT = [persist.tile([128, B * LSH], F16, tag=f"xsT{dc}",
                                name=f"xsT{dc}") for dc in range(2)]
            for dc in range(2):
                pt = ps_tp.tile([128, 128], F32, tag="tp")
                nc.tensor.transpose(pt, xs[:, dc * 128:(dc + 1) * 128], ident)
                nc.scalar.copy(out=xsT[dc], in_=pt)

            ps_l = ps_proj.tile([128, L], F32, tag="pr", name="ps_l")
            ps_l = ps_l[:, 0:B * LSH]
            for dc in range(2):
                nc.tensor.matmul(ps_l, wl_sb[dc], xsT[dc],
                                 start=(dc == 0), stop=False)
            nc.tensor.matmul(ps_l, wl_mo, m2s, start=False, stop=True)
            # leftT_r: per il row-group, columns permuted to (b, sg, q) so the
            # M_pack build's in1 column index is independent of the row group:
            # leftT_r[32il+c, b*16+sg*4+q] = left[b*64+sg*16+il*4+q, c]
            leftT = persist.tile([128, 32], F16, tag="leftT")
            for il in range(4):
                psl = slice(32 * il, 32 * il + 32)
                src = bass.AP(ps_l.tensor, ps_l[psl, il * 4:].offset,
                              [list(ps_l[psl, :].ap[0]),
                               [64, B], [16, 4], [1, 4]])
                dst = leftT[psl, :].rearrange("c (b s q) -> c b s q", b=B, q=4)
                nc.vector.tensor_copy(out=dst, in_=src)

            # ---------------- full path helper: rightT[b] [128, L] -----------
            xT = [[persist.tile([128, L], F16, tag=f"xT{b}_{dc}",
                                name=f"xT{b}_{dc}")
                   for dc in range(2)] for b in range(B)]
            rightT = [persist.tile([128, L], F16, tag=f"rt{b}",
                                   name=f"rt{b}") for b in range(B)]

            def full_path_ln(b, lc):
                t = b * 4 + lc
                xf = xf_tiles[t]
                layernorm_masked(xf, mcf_sb[:, t:t + 1])
                for dc in range(2):
                    pt = ps_tp.tile([128, 128], F32, tag="tp")
                    nc.tensor.transpose(pt, xf[:, dc * 128:(dc + 1) * 128],
                                        ident)
                    nc.scalar.copy(out=xT[b][dc][:, lc * 128:(lc + 1) * 128],
                                   in_=pt)

            def full_path_proj(b):
                ps_r = ps_proj.tile([128, L], F32, tag="pr")
                for jc in range(4):
                    jsl = slice(jc * 128, (jc + 1) * 128)
                    for dc in range(2):
                        nc.tensor.matmul(ps_r[:, jsl], wr_sb[dc],
                                         xT[b][dc][:, jsl],
                                         start=(dc == 0), stop=False)
                    nc.tensor.matmul(ps_r[:, jsl], wr_mo, m2f[b][:, jsl],
                                     start=False, stop=True)
                nc.vector.tensor_copy(out=rightT[b], in_=ps_r)

            # ---------------- M_pack builds ----------------
            # One DVE op per (b, sg): mp[32il+c, q*128+p] =
            # leftT_r[32il+c, b*16+sg*4+q] * w2[32il+c, p] via a stride-0
            # broadcast AP on the q/p free dims.
            def build_mps(b, sg):
                mp = mp_pool.tile([128, 512], F16, tag="mp",
                                  name=f"mp{b}_{sg}")
                lsl = leftT[:, b * 16 + sg * 4:]
                bc = bass.AP(lsl.tensor, lsl.offset,
                             [list(lsl.ap[0]), [1, 4], [0, 128]])
                nc.vector.tensor_tensor(
                    out=mp[:, :].rearrange("c (q x) -> c q x", x=128),
                    in0=w2_sb[:, :].rearrange("c (q x) -> c q x", x=128),
                    in1=bc, op=mybir.AluOpType.mult)
                return mp

            # ---------------- main pair loop ----------------
            COPY_PAT = "svsvsvsvsvsvsvss"   # ACT 9 : DVE 7
            copy_cnt = [0]

            def main_loop(b, sg, extra=None):
                mp = build_mps(b, sg)
                for jcp in range(2):
                    stg = stag_pool.tile([128, 4096], F16, tag="stag")
                    for jh in range(2):
                        jc = jcp * 2 + jh
                        jsl = slice(jc * 128, (jc + 1) * 128)
                        pbs = [ps_big.tile([128, 1024], F32, tag="big",
                                           name=f"pb{h2}")
                               for h2 in range(2)]
                        for il in range(4):
                            psl = slice(32 * il, 32 * il + 32)
                            nc.tensor.matmul(
                                pbs[il // 2][:, (il % 2) * 512:
                                             (il % 2 + 1) * 512],
                                rightT[b][psl, jsl], mp[psl, :],
                                start=True, stop=True,
                                tile_position=(32 * il, 0))
                        for half in range(2):
                            dst = stg[:, jh * 2048 + half * 1024:
                                      jh * 2048 + (half + 1) * 1024]
                            if COPY_PAT[copy_cnt[0] % len(COPY_PAT)] == "s":
                                nc.scalar.copy(out=dst, in_=pbs[half])
                            else:
                                nc.vector.tensor_copy(out=dst, in_=pbs[half])
                            copy_cnt[0] += 1
                    dst_ap = out[b, jcp, sg, :, :, :, :]
                    src_ap = stg[:, :].rearrange("j (jh i p) -> j jh i p",
                                                 jh=2, p=128)
                    deng = nc.sync if (sg + jcp) % 2 == 0 else nc.scalar
                    deng.dma_start(out=dst_ap, in_=src_ap)
                if extra is not None:
                    extra()

            # b=0 full path, then its main loop; b=1's LayerNorm/transpose/
            # projection work is interleaved between b=0's sg groups so the
            # PE and DVE never sit idle waiting for b=1 inputs.
            for lc in range(4):
                full_path_ln(0, lc)
            full_path_proj(0)

            b1_stages = [
                lambda: full_path_ln(1, 0),
                lambda: full_path_ln(1, 1),
                lambda: full_path_ln(1, 2),
                lambda: (full_path_ln(1, 3), full_path_proj(1)),
            ]
            for sg in range(4):
                main_loop(0, sg, extra=b1_stages[sg])
            for sg in range(4):
                main_loop(1, sg)

    nc.compile()
    names = ["node_full", "node_shard", "mask_col_full", "mask_col_shard",
             "m2_full", "m2_shard", "w_left_e", "w_right_e", "w2"]
    return nc, names


def _prepare_in_maps(node, mask, ln_gamma, ln_beta, W_left, b_left, W_right,
                     b_right, W_out, b_out):
    f = np.float32
    node = np.ascontiguousarray(np.asarray(node, dtype=f))        # [B, L, D]
    mask_f = np.asarray(mask).astype(f)                           # [B, L]
    gamma = np.asarray(ln_gamma, dtype=f)
    beta = np.asarray(ln_beta, dtype=f)
    W_l = np.asarray(W_left, dtype=f)
    W_r = np.asarray(W_right, dtype=f)
    b_l = np.asarray(b_left, dtype=f)
    b_r = np.asarray(b_right, dtype=f)
    W_o = np.asarray(W_out, dtype=f)
    b_o = np.asarray(b_out, dtype=f)

    s = 1.0 / np.sqrt(np.float32(DH))
    w_left_e = np.tile(np.concatenate(
        [gamma[:, None] * W_l, (beta @ W_l)[None, :], b_l[None, :]], 0),
        (1, 4))
    w_right_e = np.tile(np.concatenate(
        [gamma[:, None] * W_r, (beta @ W_r)[None, :], b_r[None, :]], 0),
        (1, 4)) * s
    w2 = np.tile(np.tile(np.repeat(W_o, 2, axis=0), (4, 1)), (1, 4))

    node_flat = node.reshape(B * L, D)
    mask_col_full = np.ascontiguousarray(mask_f.reshape(-1, 128).T)  # [128, 8]

    f16 = np.float16
    m2_full = np.empty((B * 2, L), dtype=f16)
    for b in range(B):
        m2_full[2 * b] = mask_f[b].astype(f16)
        m2_full[2 * b + 1] = 1.0
    common = {
        "node_full": node_flat,
        "mask_col_full": mask_col_full,
        "m2_full": m2_full,
        "w_left_e": np.ascontiguousarray(w_left_e.astype(f16)),
        "w_right_e": np.ascontiguousarray(w_right_e.astype(f16)),
        "w2": np.ascontiguousarray(w2.astype(f16)),
    }

    in_maps = []
    for c in range(NCORES):
        sl = slice(c * LSH, (c + 1) * LSH)
        shard = np.ascontiguousarray(node[:, sl, :].reshape(B * LSH, D))
        msk = mask_f[:, sl]                                       # [B, LSH]
        m = dict(common)
        m["node_shard"] = shard
        m["mask_col_shard"] = np.ascontiguousarray(msk.reshape(-1)[:, None])
        m2_sh = np.empty((2, B * LSH), dtype=f16)
        m2_sh[0] = msk.reshape(-1).astype(f16)
        m2_sh[1] = 1.0
        m["m2_shard"] = m2_sh
        in_maps.append(m)
    return in_maps


def kernel(**inputs):
    global _COMPILED
    if _COMPILED is None:
        _COMPILED = _build_program()
    nc, names = _COMPILED
    in_maps = _prepare_in_maps(**inputs)
    res = run_bass_kernel_spmd(nc, in_maps, core_ids=list(range(NCORES)))
    b_out = np.asarray(inputs["b_out"], dtype=np.float32)
    full = np.empty((B, L, L, PAIR), np.float32)
    for c in range(NCORES):
        dev = res.results[c]["out"]   # [b, jcp, sg, j, jh, i16, p] fp16
        full[:, c * LSH:(c + 1) * LSH] = (
            dev.transpose(0, 2, 5, 1, 4, 3, 6).reshape(B, LSH, L, PAIR)
            .astype(np.float32) + b_out)
    return full


if __name__ == "__main__":
    # self-test with NON-trivial gamma/beta/mask against a numpy reference
    rng = np.random.default_rng(1)
    mask = np.ones((B, L), dtype=bool)
    mask[0, 500:] = False        # exercise the mask path
    mask[1, :3] = False
    inputs = {
        "node": rng.standard_normal((B, L, D)).astype(np.float32),
        "mask": mask,
        "ln_gamma": (1.0 + 0.1 * rng.standard_normal(D)).astype(np.float32),
        "ln_beta": (0.1 * rng.standard_normal(D)).astype(np.float32),
        "W_left": (rng.standard_normal((D, DH)) / np.sqrt(D)).astype(np.float32),
        "b_left": (0.1 * rng.standard_normal(DH)).astype(np.float32),
        "W_right": (rng.standard_normal((D, DH)) / np.sqrt(D)).astype(np.float32),
        "b_right": (0.1 * rng.standard_normal(DH)).astype(np.float32),
        "W_out": (rng.standard_normal((H, PAIR)) / np.sqrt(H)).astype(np.float32),
        "b_out": (0.1 * rng.standard_normal(PAIR)).astype(np.float32),
    }

    def np_reference(node, mask, ln_gamma, ln_beta, W_left, b_left, W_right,
                     b_right, W_out, b_out):
        node = node.astype(np.float64)
        mu = node.mean(-1, keepdims=True)
        var = ((node - mu) ** 2).mean(-1, keepdims=True)
        x = (node - mu) / np.sqrt(var + LN_EPS) * ln_gamma + ln_beta
        x = x * mask[..., None]
        left = (x @ W_left + b_left).reshape(B, L, H, -1)
        right = ((x @ W_right + b_right) / np.sqrt(DH)).reshape(B, L, H, -1)
        o = np.einsum("bihk,bjhk->bijh", left, right)
        return np.einsum("bijh,hp->bijp", o, W_out) + b_out

    got = kernel(**inputs)
    exp = np_reference(**inputs)
    rel = np.abs(got - exp).max() / np.abs(exp).max()
    print("general-path rel err:", rel)
    assert rel < 5e-3, rel
    print("OK", got.shape, got.dtype)


# revision 52
# speedup vs baseline: 1.0440x; 1.0440x over previous
"""Trainium2 Bass kernel for nn_Node2Pair_bias (LayerNorm -> dual projection ->
pair outer-product -> head-mix linear).

Reference computation (B=2, L=512, D=256, DH=32, H=16, K=2, P=128):
    x   = LayerNorm(node) * gamma + beta, masked        [B, L, D]
    left  = (x @ W_left + b_left)                       [B, L, DH] -> [B,L,H,K]
    right = (x @ W_right + b_right)/sqrt(DH)            [B, L, DH] -> [B,L,H,K]
    out[b,i,j,h] = sum_k left[b,i,h,k]*right[b,j,h,k]
    out[b,i,j,p] = sum_h out[b,i,j,h]*W_out[h,p] + b_out[p]   [B, L, L, P]

Mathematical restructuring (c = (h,k) combined channel, 0..31):
    out[b,i,j,p] = sum_c right[b,j,c] * (left[b,i,c] * W2[c,p]) + b_out[p]
with W2[c,p] = W_out[c//2, p].  For each i, M_i[c,p] = left[b,i,c]*W2[c,p] is
built on a vector-class engine; 4 i's pack side by side into an rhs of
[32, 512], and the K=32 contraction uses only one 32-row group of the PE
array — so 4 consecutive i-blocks (il=0..3) are row-packed via
tile_position=(32*il, 0) and run CONCURRENTLY on disjoint row groups:
  lhsT = rightT_quad[32il:32il+32, j-chunk]   (right values, 4 replicas)
  rhs  = mp_quad[32il:32il+32, (i4, p)=512]
  -> psum_il[j=128, (i4, p)=512]
The partition-replication of rightT/leftT across the 4 row groups comes free
by tiling the projection-weight COLUMNS 4x on the host.  PSUM is drained to
fp16 staging (DVE/ACT) and DMA'd out; the host adds b_out and converts
fp16 -> f32 while un-sharding (the 2e-2 rel-err budget is ~40x the fp16
rounding error).

LayerNorm gamma/beta and both projection biases are folded into the
projection weights (exact algebra): rows = [gamma[:,None]*W; (beta@W) paired
with a mask row; b paired with a ones row].

Sharding: the i axis of L is split across the 8 cores (sequence-parallel);
each core holds its [B, 64] slice of `left` inputs plus the full `right` side
and writes a [B, 64, L, P] output shard.  No cross-device communication.
"""

import sys

sys.path.insert(0, "/opt/trn_rl_repo")

import numpy as np

import concourse.bass as bass  # noqa: F401
import concourse.mybir as mybir
import concourse.tile as tile
from concourse import bacc
from concourse.bass_utils import run_bass_kernel_spmd
from concourse.masks import make_identity

F32 = mybir.dt.float32
F16 = mybir.dt.float16

B, L, D = 2, 512, 256
DH, H, PAIR = 32, 16, 128
NCORES = 8
LSH = L // NCORES          # 64 i's per core per batch
LN_EPS = 1e-5

_COMPILED = None  # (nc, input_names)


def _build_program():
    nc = bacc.Bacc("TRN2", target_bir_lowering=False, debug=False,
                   num_devices=NCORES)

    # ---------------- DRAM parameters ----------------
    def din(name, shape, dt=F32):
        return nc.dram_tensor(name, list(shape), dt, kind="ExternalInput").ap()

    node_full = din("node_full", (B * L, D))        # all rows, (b,l) major
    node_shard = din("node_shard", (B * LSH, D))    # this core's i rows, (b,i)
    mask_col_full = din("mask_col_full", (128, B * L // 128))  # [:, t] = tile t
    mask_col_shard = din("mask_col_shard", (128, 1))
    m2_full = din("m2_full", (B * 2, L), F16)       # per b: [mask row; ones]
    m2_shard = din("m2_shard", (2, B * LSH), F16)   # [mask row; ones]
    # columns tiled 4x (col 32*r + dh = W[:, dh]) so projections emit the
    # 4-replica partition layout row-packing needs
    w_left_e = din("w_left_e", (D + 2, 4 * DH), F16)   # [gamma*W; beta@W; b_l]
    w_right_e = din("w_right_e", (D + 2, 4 * DH), F16)  # scaled by 1/sqrt(DH)
    w2 = din("w2", (4 * DH, 4 * PAIR), F16)  # quad rows, free dim tiled 4x

    # Output layout: [b, jcp, sg, j, jh, i16, p] fp16 — each 1 MiB staging
    # buffer lands as one fully contiguous partition-major stream (8 KiB per
    # partition).  Host un-permutes and upcasts while assembling the output.
    out = nc.dram_tensor("out", [B, 2, 4, 128, 2, 16, PAIR], F16,
                         kind="ExternalOutput").ap()

    NT_FULL = B * L // 128   # 8 LayerNorm tiles for the full sequence

    with tile.TileContext(nc) as tc:
        with (
            tc.tile_pool(name="singles", bufs=1) as singles,
            tc.tile_pool(name="xpool", bufs=9) as xpool,
            tc.tile_pool(name="stats", bufs=4) as stats,
            tc.tile_pool(name="persist", bufs=1) as persist,
            tc.tile_pool(name="mp", bufs=4) as mp_pool,
            tc.tile_pool(name="stag", bufs=6) as stag_pool,
            tc.tile_pool(name="ps_tp", bufs=1, space="PSUM") as ps_tp,
            tc.tile_pool(name="ps_proj", bufs=1, space="PSUM") as ps_proj,
            tc.tile_pool(name="ps_big", bufs=3, space="PSUM") as ps_big,
        ):
            # ---------------- constants ----------------
            ident = singles.tile([128, 128], F32, tag="ident")
            make_identity(nc, ident)
            eps_t = singles.tile([128, 1], F32, tag="eps")
            nc.vector.memset(eps_t, LN_EPS)

            # hot-path loads on sync (HWDGE) in dependency-critical order
            # (small weights first so nothing queues behind the node tiles);
            # b=1 tiles + right-side constants via gpsimd (SWDGE) in parallel
            xs = xpool.tile([128, D], F32, tag="x", name="xs")
            nc.sync.dma_start(out=xs, in_=node_shard[:, :])
            mcs_sb = singles.tile([128, 1], F32, tag="mcs")
            nc.sync.dma_start(out=mcs_sb, in_=mask_col_shard[:, :])
            wl_sb = [singles.tile([128, 4 * DH], F16, tag=f"wl{dc}",
                                  name=f"wl{dc}") for dc in range(2)]
            for dc in range(2):
                nc.sync.dma_start(out=wl_sb[dc],
                                  in_=w_left_e[dc * 128:(dc + 1) * 128, :])
            wl_mo = singles.tile([2, 4 * DH], F16, tag="wlmo")
            nc.sync.dma_start(out=wl_mo, in_=w_left_e[D:D + 2, :])
            w2_sb = singles.tile([4 * DH, 4 * PAIR], F16, tag="w2")
            nc.sync.dma_start(out=w2_sb, in_=w2[:, :])
            # shard mask+ones rhs rows [2, B*LSH]
            m2s = singles.tile([2, B * LSH], F16, tag="m2s")
            nc.sync.dma_start(out=m2s, in_=m2_shard[:, :])
            # b=0 node tiles + mask on the scalar HWDGE queue — the ACT
            # sequencer is idle this early, so these land in parallel with
            # the sync queue's weight loads instead of queuing behind them
            mcf_sb = singles.tile([128, NT_FULL], F32, tag="mcf")
            nc.scalar.dma_start(out=mcf_sb, in_=mask_col_full[:, :])
            xf_tiles = [None] * NT_FULL
            for t in range(4):
                xf = xpool.tile([128, D], F32, tag="x", name=f"xf{t}")
                nc.scalar.dma_start(out=xf,
                                    in_=node_full[t * 128:(t + 1) * 128, :])
                xf_tiles[t] = xf

            wr_sb = [singles.tile([128, 4 * DH], F16, tag=f"wr{dc}",
                                  name=f"wr{dc}") for dc in range(2)]
            for dc in range(2):
                nc.gpsimd.dma_start(out=wr_sb[dc],
                                    in_=w_right_e[dc * 128:(dc + 1) * 128, :])
            wr_mo = singles.tile([2, 4 * DH], F16, tag="wrmo")
            nc.gpsimd.dma_start(out=wr_mo, in_=w_right_e[D:D + 2, :])
            m2f = [singles.tile([2, L], F16, tag=f"m2f{b}", name=f"m2f{b}")
                   for b in range(B)]
            for b in range(B):
                nc.gpsimd.dma_start(out=m2f[b],
                                    in_=m2_full[2 * b:2 * b + 2, :])
            for t in range(4, NT_FULL):
                xf = xpool.tile([128, D], F32, tag="x", name=f"xf{t}")
                nc.gpsimd.dma_start(out=xf,
                                    in_=node_full[t * 128:(t + 1) * 128, :])
                xf_tiles[t] = xf

            # ---------------- LayerNorm helper ----------------
            def layernorm_masked(x_t, mask_col_ap, apply_eng=None):
                """x_t [128, D] in place -> (x - mu) * rsqrt(var+eps) * mask."""
                st = stats.tile([128, 6], F32, tag="st")
                nc.vector.bn_stats(out=st, in_=x_t)
                mv = stats.tile([128, 2], F32, tag="mv")
                nc.vector.bn_aggr(out=mv, in_=st)
                sd = stats.tile([128, 1], F32, tag="sd")
                nc.scalar.activation(out=sd, in_=mv[:, 1:2],
                                     func=mybir.ActivationFunctionType.Sqrt,
                                     bias=eps_t, scale=1.0)
                rs = stats.tile([128, 1], F32, tag="rs")
                nc.vector.reciprocal(out=rs, in_=sd)
                rsm = stats.tile([128, 1], F32, tag="rsm")
                nc.vector.tensor_mul(out=rsm, in0=rs, in1=mask_col_ap)
                (apply_eng or nc.vector).tensor_scalar(
                    out=x_t, in0=x_t,
                    scalar1=mv[:, 0:1], scalar2=rsm,
                    op0=mybir.AluOpType.subtract,
                    op1=mybir.AluOpType.mult)

            # ---------------- shard path: leftT_r [128, B*LSH] ---------------
            layernorm_masked(xs, mcs_sb[:, 0:1])

            xsT = [persist.tile([128, B * LSH], F16, tag=f"xsT{dc}",
                                name=f"xsT{dc}") for dc in range(2)]
            for dc in range(2):
                pt = ps_tp.tile([128, 128], F32, tag="tp")
                nc.tensor.transpose(pt, xs[:, dc * 128:(dc + 1) * 128], ident)
                nc.scalar.copy(out=xsT[dc], in_=pt)

            ps_l = ps_proj.tile([128, L], F32, tag="pr", name="ps_l")
            ps_l = ps_l[:, 0:B * LSH]
            for dc in range(2):
                nc.tensor.matmul(ps_l, wl_sb[dc], xsT[dc],
                                 start=(dc == 0), stop=False)
            nc.tensor.matmul(ps_l, wl_mo, m2s, start=False, stop=True)
            # leftT_r: per il row-group, columns permuted to (b, sg, q) so the
            # M_pack build's in1 column index is independent of the row group:
            # leftT_r[32il+c, b*16+sg*4+q] = left[b*64+sg*16+il*4+q, c]
            leftT = persist.tile([128, 32], F16, tag="leftT")
            for il in range(4):
                psl = slice(32 * il, 32 * il + 32)
                src = bass.AP(ps_l.tensor, ps_l[psl, il * 4:].offset,
                              [list(ps_l[psl, :].ap[0]),
                               [64, B], [16, 4], [1, 4]])
                dst = leftT[psl, :].rearrange("c (b s q) -> c b s q", b=B, q=4)
                nc.vector.tensor_copy(out=dst, in_=src)

            # ---------------- full path helper: rightT[b] [128, L] -----------
            xT = [[persist.tile([128, L], F16, tag=f"xT{b}_{dc}",
                                name=f"xT{b}_{dc}")
                   for dc in range(2)] for b in range(B)]
            rightT = [persist.tile([128, L], F16, tag=f"rt{b}",
                                   name=f"rt{b}") for b in range(B)]

            def full_path_ln(b, lc):
                t = b * 4 + lc
                xf = xf_tiles[t]
                layernorm_masked(xf, mcf_sb[:, t:t + 1],
                                 apply_eng=nc.gpsimd if b == 1 else None)
                for dc in range(2):
                    pt = ps_tp.tile([128, 128], F32, tag="tp")
                    nc.tensor.transpose(pt, xf[:, dc * 128:(dc + 1) * 128],
                                        ident)
                    nc.scalar.copy(out=xT[b][dc][:, lc * 128:(lc + 1) * 128],
                                   in_=pt)

            def full_path_proj(b):
                ps_r = ps_proj.tile([128, L], F32, tag="pr")
                for jc in range(4):
                    jsl = slice(jc * 128, (jc + 1) * 128)
                    for dc in range(2):
                        nc.tensor.matmul(ps_r[:, jsl], wr_sb[dc],
                                         xT[b][dc][:, jsl],
                                         start=(dc == 0), stop=False)
                    nc.tensor.matmul(ps_r[:, jsl], wr_mo, m2f[b][:, jsl],
                                     start=False, stop=True)
                nc.vector.tensor_copy(out=rightT[b], in_=ps_r)

            # ---------------- M_pack builds ----------------
            # One DVE op per (b, sg): mp[32il+c, q*128+p] =
            # leftT_r[32il+c, b*16+sg*4+q] * w2[32il+c, p] via a stride-0
            # broadcast AP on the q/p free dims.
            def build_mps(b, sg):
                mp = mp_pool.tile([128, 512], F16, tag="mp",
                                  name=f"mp{b}_{sg}")
                lsl = leftT[:, b * 16 + sg * 4:]
                bc = bass.AP(lsl.tensor, lsl.offset,
                             [list(lsl.ap[0]), [1, 4], [0, 128]])
                nc.vector.tensor_tensor(
                    out=mp[:, :].rearrange("c (q x) -> c q x", x=128),
                    in0=w2_sb[:, :].rearrange("c (q x) -> c q x", x=128),
                    in1=bc, op=mybir.AluOpType.mult)
                return mp

            # ---------------- main pair loop ----------------
            COPY_PAT = "svsvsvsvsvsvsvss"   # ACT 9 : DVE 7
            copy_cnt = [0]

            def main_loop(b, sg, extra=None, fine_dma=False):
                mp = build_mps(b, sg)
                for jcp in range(2):
                    stg = stag_pool.tile([128, 4096], F16, tag="stag")
                    for jh in range(2):
                        jc = jcp * 2 + jh
                        jsl = slice(jc * 128, (jc + 1) * 128)
                        pbs = [ps_big.tile([128, 1024], F32, tag="big",
                                           name=f"pb{h2}")
                               for h2 in range(2)]
                        for il in range(4):
                            psl = slice(32 * il, 32 * il + 32)
                            nc.tensor.matmul(
                                pbs[il // 2][:, (il % 2) * 512:
                                             (il % 2 + 1) * 512],
                                rightT[b][psl, jsl], mp[psl, :],
                                start=True, stop=True,
                                tile_position=(32 * il, 0))
                        for half in range(2):
                            dst = stg[:, jh * 2048 + half * 1024:
                                      jh * 2048 + (half + 1) * 1024]
                            if COPY_PAT[copy_cnt[0] % len(COPY_PAT)] == "s":
                                nc.scalar.copy(out=dst, in_=pbs[half])
                            else:
                                nc.vector.tensor_copy(out=dst, in_=pbs[half])
                            copy_cnt[0] += 1
                    if fine_dma:
                        for jh in range(2):
                            dst_ap = out[b, jcp, sg, :, jh, :, :]
                            src_ap = stg[:, jh * 2048:(jh + 1) * 2048]\
                                .rearrange("j (i p) -> j i p", p=128)
                            nc.sync.dma_start(out=dst_ap, in_=src_ap)
                    else:
                        dst_ap = out[b, jcp, sg, :, :, :, :]
                        src_ap = stg[:, :].rearrange(
                            "j (jh i p) -> j jh i p", jh=2, p=128)
                        nc.sync.dma_start(out=dst_ap, in_=src_ap)
                if extra is not None:
                    extra()

            # b=0 full path, then its main loop; b=1's LayerNorm/transpose/
            # projection work is interleaved between b=0's sg groups so the
            # PE and DVE never sit idle waiting for b=1 inputs.
            for lc in range(4):
                full_path_ln(0, lc)
            full_path_proj(0)

            b1_stages = [
                lambda: full_path_ln(1, 0),
                lambda: full_path_ln(1, 1),
                lambda: full_path_ln(1, 2),
                lambda: (full_path_ln(1, 3), full_path_proj(1)),
            ]
            for sg in range(4):
                main_loop(0, sg, extra=b1_stages[sg], fine_dma=(sg == 0))
            for sg in range(4):
                main_loop(1, sg, fine_dma=(sg == 3))

    nc.compile()
    names = ["node_full", "node_shard", "mask_col_full", "mask_col_shard",
             "m2_full", "m2_shard", "w_left_e", "w_right_e", "w2"]
    return nc, names


def _prepare_in_maps(node, mask, ln_gamma, ln_beta, W_left, b_left, W_right,
                     b_right, W_out, b_out):
    f = np.float32
    node = np.ascontiguousarray(np.asarray(node, dtype=f))        # [B, L, D]
    mask_f = np.asarray(mask).astype(f)                           # [B, L]
    gamma = np.asarray(ln_gamma, dtype=f)
    beta = np.asarray(ln_beta, dtype=f)
    W_l = np.asarray(W_left, dtype=f)
    W_r = np.asarray(W_right, dtype=f)
    b_l = np.asarray(b_left, dtype=f)
    b_r = np.asarray(b_right, dtype=f)
    W_o = np.asarray(W_out, dtype=f)
    b_o = np.asarray(b_out, dtype=f)

    s = 1.0 / np.sqrt(np.float32(DH))
    w_left_e = np.tile(np.concatenate(
        [gamma[:, None] * W_l, (beta @ W_l)[None, :], b_l[None, :]], 0),
        (1, 4))
    w_right_e = np.tile(np.concatenate(
        [gamma[:, None] * W_r, (beta @ W_r)[None, :], b_r[None, :]], 0),
        (1, 4)) * s
    w2 = np.tile(np.tile(np.repeat(W_o, 2, axis=0), (4, 1)), (1, 4))

    node_flat = node.reshape(B * L, D)
    mask_col_full = np.ascontiguousarray(mask_f.reshape(-1, 128).T)  # [128, 8]

    f16 = np.float16
    m2_full = np.empty((B * 2, L), dtype=f16)
    for b in range(B):
        m2_full[2 * b] = mask_f[b].astype(f16)
        m2_full[2 * b + 1] = 1.0
    common = {
        "node_full": node_flat,
        "mask_col_full": mask_col_full,
        "m2_full": m2_full,
        "w_left_e": np.ascontiguousarray(w_left_e.astype(f16)),
        "w_right_e": np.ascontiguousarray(w_right_e.astype(f16)),
        "w2": np.ascontiguousarray(w2.astype(f16)),
    }

    in_maps = []
    for c in range(NCORES):
        sl = slice(c * LSH, (c + 1) * LSH)
        shard = np.ascontiguousarray(node[:, sl, :].reshape(B * LSH, D))
        msk = mask_f[:, sl]                                       # [B, LSH]
        m = dict(common)
        m["node_shard"] = shard
        m["mask_col_shard"] = np.ascontiguousarray(msk.reshape(-1)[:, None])
        m2_sh = np.empty((2, B * LSH), dtype=f16)
        m2_sh[0] = msk.reshape(-1).astype(f16)
        m2_sh[1] = 1.0
        m["m2_shard"] = m2_sh
        in_maps.append(m)
    return in_maps


def kernel(**inputs):
    global _COMPILED
    if _COMPILED is None:
        _COMPILED = _build_program()
    nc, names = _COMPILED
    in_maps = _prepare_in_maps(**inputs)
    res = run_bass_kernel_spmd(nc, in_maps, core_ids=list(range(NCORES)))
    b_out = np.asarray(inputs["b_out"], dtype=np.float32)
    full = np.empty((B, L, L, PAIR), np.float32)
    for c in range(NCORES):
        dev = res.results[c]["out"]   # [b, jcp, sg, j, jh, i16, p] fp16
        full[:, c * LSH:(c + 1) * LSH] = (
            dev.transpose(0, 2, 5, 1, 4, 3, 6).reshape(B, LSH, L, PAIR)
            .astype(np.float32) + b_out)
    return full


if __name__ == "__main__":
    # self-test with NON-trivial gamma/beta/mask against a numpy reference
    rng = np.random.default_rng(1)
    mask = np.ones((B, L), dtype=bool)
    mask[0, 500:] = False        # exercise the mask path
    mask[1, :3] = False
    inputs = {
        "node": rng.standard_normal((B, L, D)).astype(np.float32),
        "mask": mask,
        "ln_gamma": (1.0 + 0.1 * rng.standard_normal(D)).astype(np.float32),
        "ln_beta": (0.1 * rng.standard_normal(D)).astype(np.float32),
        "W_left": (rng.standard_normal((D, DH)) / np.sqrt(D)).astype(np.float32),
        "b_left": (0.1 * rng.standard_normal(DH)).astype(np.float32),
        "W_right": (rng.standard_normal((D, DH)) / np.sqrt(D)).astype(np.float32),
        "b_right": (0.1 * rng.standard_normal(DH)).astype(np.float32),
        "W_out": (rng.standard_normal((H, PAIR)) / np.sqrt(H)).astype(np.float32),
        "b_out": (0.1 * rng.standard_normal(PAIR)).astype(np.float32),
    }

    def np_reference(node, mask, ln_gamma, ln_beta, W_left, b_left, W_right,
                     b_right, W_out, b_out):
        node = node.astype(np.float64)
        mu = node.mean(-1, keepdims=True)
        var = ((node - mu) ** 2).mean(-1, keepdims=True)
        x = (node - mu) / np.sqrt(var + LN_EPS) * ln_gamma + ln_beta
        x = x * mask[..., None]
        left = (x @ W_left + b_left).reshape(B, L, H, -1)
        right = ((x @ W_right + b_right) / np.sqrt(DH)).reshape(B, L, H, -1)
        o = np.einsum("bihk,bjhk->bijh", left, right)
        return np.einsum("bijh,hp->bijp", o, W_out) + b_out

    got = kernel(**inputs)
    exp = np_reference(**inputs)
    rel = np.abs(got - exp).max() / np.abs(exp).max()
    print("general-path rel err:", rel)
    assert rel < 5e-3, rel
    print("OK", got.shape, got.dtype)


# revision 53
# speedup vs baseline: 1.0820x; 1.0363x over previous
"""Trainium2 Bass kernel for nn_Node2Pair_bias (LayerNorm -> dual projection ->
pair outer-product -> head-mix linear).

Reference computation (B=2, L=512, D=256, DH=32, H=16, K=2, P=128):
    x   = LayerNorm(node) * gamma + beta, masked        [B, L, D]
    left  = (x @ W_left + b_left)                       [B, L, DH] -> [B,L,H,K]
    right = (x @ W_right + b_right)/sqrt(DH)            [B, L, DH] -> [B,L,H,K]
    out[b,i,j,h] = sum_k left[b,i,h,k]*right[b,j,h,k]
    out[b,i,j,p] = sum_h out[b,i,j,h]*W_out[h,p] + b_out[p]   [B, L, L, P]

Mathematical restructuring (c = (h,k) combined channel, 0..31):
    out[b,i,j,p] = sum_c right[b,j,c] * (left[b,i,c] * W2[c,p]) + b_out[p]
with W2[c,p] = W_out[c//2, p].  For each i, M_i[c,p] = left[b,i,c]*W2[c,p] is
built on a vector-class engine; 4 i's pack side by side into an rhs of
[32, 512], and the K=32 contraction uses only one 32-row group of the PE
array — so 4 consecutive i-blocks (il=0..3) are row-packed via
tile_position=(32*il, 0) and run CONCURRENTLY on disjoint row groups:
  lhsT = rightT_quad[32il:32il+32, j-chunk]   (right values, 4 replicas)
  rhs  = mp_quad[32il:32il+32, (i4, p)=512]
  -> psum_il[j=128, (i4, p)=512]
The partition-replication of rightT/leftT across the 4 row groups comes free
by tiling the projection-weight COLUMNS 4x on the host.  PSUM is drained to
fp16 staging (DVE/ACT) and DMA'd out; the host adds b_out and converts
fp16 -> f32 while un-sharding (the 2e-2 rel-err budget is ~40x the fp16
rounding error).

LayerNorm gamma/beta and both projection biases are folded into the
projection weights (exact algebra): rows = [gamma[:,None]*W; (beta@W) paired
with a mask row; b paired with a ones row].

Sharding: the i axis of L is split across the 8 cores (sequence-parallel);
each core holds its [B, 64] slice of `left` inputs plus the full `right` side
and writes a [B, 64, L, P] output shard.  No cross-device communication.
"""

import sys

sys.path.insert(0, "/opt/trn_rl_repo")

import numpy as np

import concourse.bass as bass  # noqa: F401
import concourse.mybir as mybir
import concourse.tile as tile
from concourse import bacc
from concourse.bass_utils import run_bass_kernel_spmd
from concourse.masks import make_identity

F32 = mybir.dt.float32
F16 = mybir.dt.float16

B, L, D = 2, 512, 256
DH, H, PAIR = 32, 16, 128
NCORES = 8
LSH = L // NCORES          # 64 i's per core per batch
LN_EPS = 1e-5

_COMPILED = None  # (nc, input_names)


def _build_program():
    nc = bacc.Bacc("TRN2", target_bir_lowering=False, debug=False,
                   num_devices=NCORES)

    # ---------------- DRAM parameters ----------------
    def din(name, shape, dt=F32):
        return nc.dram_tensor(name, list(shape), dt, kind="ExternalInput").ap()

    node_full = din("node_full", (B * L, D))        # all rows, (b,l) major
    node_shard = din("node_shard", (B * LSH, D))    # this core's i rows, (b,i)
    mask_col_full = din("mask_col_full", (128, B * L // 128))  # [:, t] = tile t
    mask_col_shard = din("mask_col_shard", (128, 1))
    m2_full = din("m2_full", (B * 2, L), F16)       # per b: [mask row; ones]
    m2_shard = din("m2_shard", (2, B * LSH), F16)   # [mask row; ones]
    # columns tiled 4x (col 32*r + dh = W[:, dh]) so projections emit the
    # 4-replica partition layout row-packing needs
    w_left_e = din("w_left_e", (D + 2, 4 * DH), F16)   # [gamma*W; beta@W; b_l]
    w_right_e = din("w_right_e", (D + 2, 4 * DH), F16)  # scaled by 1/sqrt(DH)
    w2 = din("w2", (4 * DH, 4 * PAIR), F16)  # quad rows, free dim tiled 4x

    # Output layout: [b, jcp, sg, j, jh, i16, p] fp16 — each 1 MiB staging
    # buffer lands as one fully contiguous partition-major stream (8 KiB per
    # partition).  Host un-permutes and upcasts while assembling the output.
    out = nc.dram_tensor("out", [B, 2, 4, 128, 2, 16, PAIR], F16,
                         kind="ExternalOutput").ap()

    NT_FULL = B * L // 128   # 8 LayerNorm tiles for the full sequence

    with tile.TileContext(nc) as tc:
        with (
            tc.tile_pool(name="singles", bufs=1) as singles,
            tc.tile_pool(name="xpool", bufs=9) as xpool,
            tc.tile_pool(name="stats", bufs=4) as stats,
            tc.tile_pool(name="persist", bufs=1) as persist,
            tc.tile_pool(name="mp", bufs=4) as mp_pool,
            tc.tile_pool(name="stag", bufs=6) as stag_pool,
            tc.tile_pool(name="ps_tp", bufs=1, space="PSUM") as ps_tp,
            tc.tile_pool(name="ps_proj", bufs=1, space="PSUM") as ps_proj,
            tc.tile_pool(name="ps_big", bufs=3, space="PSUM") as ps_big,
        ):
            # ---------------- constants ----------------
            ident = singles.tile([128, 128], F32, tag="ident")
            make_identity(nc, ident)
            eps_t = singles.tile([128, 1], F32, tag="eps")
            nc.vector.memset(eps_t, LN_EPS)

            # hot-path loads on sync (HWDGE) in dependency-critical order
            # (small weights first so nothing queues behind the node tiles);
            # b=1 tiles + right-side constants via gpsimd (SWDGE) in parallel
            xs = xpool.tile([128, D], F32, tag="x", name="xs")
            nc.sync.dma_start(out=xs, in_=node_shard[:, :])
            mcs_sb = singles.tile([128, 1], F32, tag="mcs")
            nc.sync.dma_start(out=mcs_sb, in_=mask_col_shard[:, :])
            wl_sb = [singles.tile([128, 4 * DH], F16, tag=f"wl{dc}",
                                  name=f"wl{dc}") for dc in range(2)]
            for dc in range(2):
                nc.sync.dma_start(out=wl_sb[dc],
                                  in_=w_left_e[dc * 128:(dc + 1) * 128, :])
            wl_mo = singles.tile([2, 4 * DH], F16, tag="wlmo")
            nc.sync.dma_start(out=wl_mo, in_=w_left_e[D:D + 2, :])
            w2_sb = singles.tile([4 * DH, 4 * PAIR], F16, tag="w2")
            nc.sync.dma_start(out=w2_sb, in_=w2[:, :])
            # shard mask+ones rhs rows [2, B*LSH]
            m2s = singles.tile([2, B * LSH], F16, tag="m2s")
            nc.sync.dma_start(out=m2s, in_=m2_shard[:, :])
            # b=0 node tiles + mask on the scalar HWDGE queue — the ACT
            # sequencer is idle this early, so these land in parallel with
            # the sync queue's weight loads instead of queuing behind them
            mcf_sb = singles.tile([128, NT_FULL], F32, tag="mcf")
            nc.scalar.dma_start(out=mcf_sb, in_=mask_col_full[:, :])
            xf_tiles = [None] * NT_FULL
            for t in range(4):
                xf = xpool.tile([128, D], F32, tag="x", name=f"xf{t}")
                nc.scalar.dma_start(out=xf,
                                    in_=node_full[t * 128:(t + 1) * 128, :])
                xf_tiles[t] = xf

            wr_sb = [singles.tile([128, 4 * DH], F16, tag=f"wr{dc}",
                                  name=f"wr{dc}") for dc in range(2)]
            for dc in range(2):
                nc.gpsimd.dma_start(out=wr_sb[dc],
                                    in_=w_right_e[dc * 128:(dc + 1) * 128, :])
            wr_mo = singles.tile([2, 4 * DH], F16, tag="wrmo")
            nc.gpsimd.dma_start(out=wr_mo, in_=w_right_e[D:D + 2, :])
            m2f = [singles.tile([2, L], F16, tag=f"m2f{b}", name=f"m2f{b}")
                   for b in range(B)]
            for b in range(B):
                nc.gpsimd.dma_start(out=m2f[b],
                                    in_=m2_full[2 * b:2 * b + 2, :])
            for t in range(4, NT_FULL):
                xf = xpool.tile([128, D], F32, tag="x", name=f"xf{t}")
                nc.gpsimd.dma_start(out=xf,
                                    in_=node_full[t * 128:(t + 1) * 128, :])
                xf_tiles[t] = xf

            # ---------------- LayerNorm helper ----------------
            def layernorm_masked(x_t, mask_col_ap):
                """x_t [128, D] in place -> (x - mu) * rsqrt(var+eps) * mask."""
                st = stats.tile([128, 6], F32, tag="st")
                nc.vector.bn_stats(out=st, in_=x_t)
                mv = stats.tile([128, 2], F32, tag="mv")
                nc.vector.bn_aggr(out=mv, in_=st)
                sd = stats.tile([128, 1], F32, tag="sd")
                nc.scalar.activation(out=sd, in_=mv[:, 1:2],
                                     func=mybir.ActivationFunctionType.Sqrt,
                                     bias=eps_t, scale=1.0)
                rs = stats.tile([128, 1], F32, tag="rs")
                nc.vector.reciprocal(out=rs, in_=sd)
                rsm = stats.tile([128, 1], F32, tag="rsm")
                nc.vector.tensor_mul(out=rsm, in0=rs, in1=mask_col_ap)
                nc.vector.tensor_scalar(out=x_t, in0=x_t,
                                        scalar1=mv[:, 0:1], scalar2=rsm,
                                        op0=mybir.AluOpType.subtract,
                                        op1=mybir.AluOpType.mult)

            # ---------------- shard path: leftT_r [128, B*LSH] ---------------
            layernorm_masked(xs, mcs_sb[:, 0:1])

            xsT = [persist.tile([128, B * LSH], F16, tag=f"xsT{dc}",
                                name=f"xsT{dc}") for dc in range(2)]
            for dc in range(2):
                pt = ps_tp.tile([128, 128], F32, tag="tp")
                nc.tensor.transpose(pt, xs[:, dc * 128:(dc + 1) * 128], ident)
                nc.scalar.copy(out=xsT[dc], in_=pt)

            ps_l = ps_proj.tile([128, L], F32, tag="pr", name="ps_l")
            ps_l = ps_l[:, 0:B * LSH]
            for dc in range(2):
                nc.tensor.matmul(ps_l, wl_sb[dc], xsT[dc],
                                 start=(dc == 0), stop=False)
            nc.tensor.matmul(ps_l, wl_mo, m2s, start=False, stop=True)
            # leftT_r: per il row-group, columns permuted to (b, sg, q) so the
            # M_pack build's in1 column index is independent of the row group:
            # leftT_r[32il+c, b*16+sg*4+q] = left[b*64+sg*16+il*4+q, c]
            leftT = persist.tile([128, 32], F16, tag="leftT")
            for il in range(4):
                psl = slice(32 * il, 32 * il + 32)
                src = bass.AP(ps_l.tensor, ps_l[psl, il * 4:].offset,
                              [list(ps_l[psl, :].ap[0]),
                               [64, B], [16, 4], [1, 4]])
                dst = leftT[psl, :].rearrange("c (b s q) -> c b s q", b=B, q=4)
                nc.vector.tensor_copy(out=dst, in_=src)

            # ---------------- full path helper: rightT[b] [128, L] -----------
            xT = [[persist.tile([128, L], F16, tag=f"xT{b}_{dc}",
                                name=f"xT{b}_{dc}")
                   for dc in range(2)] for b in range(B)]
            rightT = [persist.tile([128, L], F16, tag=f"rt{b}",
                                   name=f"rt{b}") for b in range(B)]

            def full_path_ln(b, lc):
                t = b * 4 + lc
                xf = xf_tiles[t]
                layernorm_masked(xf, mcf_sb[:, t:t + 1])
                for dc in range(2):
                    pt = ps_tp.tile([128, 128], F32, tag="tp")
                    nc.tensor.transpose(pt, xf[:, dc * 128:(dc + 1) * 128],
                                        ident)
                    nc.scalar.copy(out=xT[b][dc][:, lc * 128:(lc + 1) * 128],
                                   in_=pt)

            def full_path_proj(b):
                ps_r = ps_proj.tile([128, L], F32, tag="pr")
                for jc in range(4):
                    jsl = slice(jc * 128, (jc + 1) * 128)
                    for dc in range(2):
                        nc.tensor.matmul(ps_r[:, jsl], wr_sb[dc],
                                         xT[b][dc][:, jsl],
                                         start=(dc == 0), stop=False)
                    nc.tensor.matmul(ps_r[:, jsl], wr_mo, m2f[b][:, jsl],
                                     start=False, stop=True)
                nc.vector.tensor_copy(out=rightT[b], in_=ps_r)

            # ---------------- M_pack builds ----------------
            # One DVE op per (b, sg): mp[32il+c, q*128+p] =
            # leftT_r[32il+c, b*16+sg*4+q] * w2[32il+c, p] via a stride-0
            # broadcast AP on the q/p free dims.
            def build_mps(b, sg):
                mp = mp_pool.tile([128, 512], F16, tag="mp",
                                  name=f"mp{b}_{sg}")
                lsl = leftT[:, b * 16 + sg * 4:]
                bc = bass.AP(lsl.tensor, lsl.offset,
                             [list(lsl.ap[0]), [1, 4], [0, 128]])
                nc.vector.tensor_tensor(
                    out=mp[:, :].rearrange("c (q x) -> c q x", x=128),
                    in0=w2_sb[:, :].rearrange("c (q x) -> c q x", x=128),
                    in1=bc, op=mybir.AluOpType.mult)
                return mp

            # ---------------- main pair loop ----------------
            COPY_PAT = "svsvsvsvsvsvsvss"   # ACT 9 : DVE 7
            copy_cnt = [0]

            def main_loop(b, sg, extra=None):
                mp = build_mps(b, sg)
                for jcp in range(2):
                    stg = stag_pool.tile([128, 4096], F16, tag="stag")
                    for jh in range(2):
                        jc = jcp * 2 + jh
                        jsl = slice(jc * 128, (jc + 1) * 128)
                        pbs = [ps_big.tile([128, 1024], F32, tag="big",
                                           name=f"pb{h2}")
                               for h2 in range(2)]
                        for il in range(4):
                            psl = slice(32 * il, 32 * il + 32)
                            nc.tensor.matmul(
                                pbs[il // 2][:, (il % 2) * 512:
                                             (il % 2 + 1) * 512],
                                rightT[b][psl, jsl], mp[psl, :],
                                start=True, stop=True,
                                tile_position=(32 * il, 0))
                        for half in range(2):
                            dst = stg[:, jh * 2048 + half * 1024:
                                      jh * 2048 + (half + 1) * 1024]
                            if COPY_PAT[copy_cnt[0] % len(COPY_PAT)] == "s":
                                nc.scalar.copy(out=dst, in_=pbs[half])
                            else:
                                nc.vector.tensor_copy(out=dst, in_=pbs[half])
                            copy_cnt[0] += 1
                    dst_ap = out[b, jcp, sg, :, :, :, :]
                    src_ap = stg[:, :].rearrange("j (jh i p) -> j jh i p",
                                                 jh=2, p=128)
                    nc.sync.dma_start(out=dst_ap, in_=src_ap)
                if extra is not None:
                    extra()

            # b=0 full path, then its main loop; b=1's LayerNorm/transpose/
            # projection work is interleaved between b=0's sg groups so the
            # PE and DVE never sit idle waiting for b=1 inputs.
            for lc in range(4):
                full_path_ln(0, lc)
            full_path_proj(0)

            b1_stages = [
                lambda: full_path_ln(1, 0),
                lambda: full_path_ln(1, 1),
                lambda: full_path_ln(1, 2),
                lambda: (full_path_ln(1, 3), full_path_proj(1)),
            ]
            for sg in range(4):
                main_loop(0, sg, extra=b1_stages[sg])
            for sg in range(4):
                main_loop(1, sg)

    nc.compile()
    names = ["node_full", "node_shard", "mask_col_full", "mask_col_shard",
             "m2_full", "m2_shard", "w_left_e", "w_right_e", "w2"]
    return nc, names


def _prepare_in_maps(node, mask, ln_gamma, ln_beta, W_left, b_left, W_right,
                     b_right, W_out, b_out):
    f = np.float32
    node = np.ascontiguousarray(np.asarray(node, dtype=f))        # [B, L, D]
    mask_f = np.asarray(mask).astype(f)                           # [B, L]
    gamma = np.asarray(ln_gamma, dtype=f)
    beta = np.asarray(ln_beta, dtype=f)
    W_l = np.asarray(W_left, dtype=f)
    W_r = np.asarray(W_right, dtype=f)
    b_l = np.asarray(b_left, dtype=f)
    b_r = np.asarray(b_right, dtype=f)
    W_o = np.asarray(W_out, dtype=f)
    b_o = np.asarray(b_out, dtype=f)

    s = 1.0 / np.sqrt(np.float32(DH))
    w_left_e = np.tile(np.concatenate(
        [gamma[:, None] * W_l, (beta @ W_l)[None, :], b_l[None, :]], 0),
        (1, 4))
    w_right_e = np.tile(np.concatenate(
        [gamma[:, None] * W_r, (beta @ W_r)[None, :], b_r[None, :]], 0),
        (1, 4)) * s
    w2 = np.tile(np.tile(np.repeat(W_o, 2, axis=0), (4, 1)), (1, 4))

    node_flat = node.reshape(B * L, D)
    mask_col_full = np.ascontiguousarray(mask_f.reshape(-1, 128).T)  # [128, 8]

    f16 = np.float16
    m2_full = np.empty((B * 2, L), dtype=f16)
    for b in range(B):
        m2_full[2 * b] = mask_f[b].astype(f16)
        m2_full[2 * b + 1] = 1.0
    common = {
        "node_full": node_flat,
        "mask_col_full": mask_col_full,
        "m2_full": m2_full,
        "w_left_e": np.ascontiguousarray(w_left_e.astype(f16)),
        "w_right_e": np.ascontiguousarray(w_right_e.astype(f16)),
        "w2": np.ascontiguousarray(w2.astype(f16)),
    }

    in_maps = []
    for c in range(NCORES):
        sl = slice(c * LSH, (c + 1) * LSH)
        shard = np.ascontiguousarray(node[:, sl, :].reshape(B * LSH, D))
        msk = mask_f[:, sl]                                       # [B, LSH]
        m = dict(common)
        m["node_shard"] = shard
        m["mask_col_shard"] = np.ascontiguousarray(msk.reshape(-1)[:, None])
        m2_sh = np.empty((2, B * LSH), dtype=f16)
        m2_sh[0] = msk.reshape(-1).astype(f16)
        m2_sh[1] = 1.0
        m["m2_shard"] = m2_sh
        in_maps.append(m)
    return in_maps


def kernel(**inputs):
    global _COMPILED
    if _COMPILED is None:
        _COMPILED = _build_program()
    nc, names = _COMPILED
    in_maps = _prepare_in_maps(**inputs)
    res = run_bass_kernel_spmd(nc, in_maps, core_ids=list(range(NCORES)))
    b_out = np.asarray(inputs["b_out"], dtype=np.float32)
    full = np.empty((B, L, L, PAIR), np.float32)
    for c in range(NCORES):
        dev = res.results[c]["out"]   # [b, jcp, sg, j, jh, i16, p] fp16
        full[:, c * LSH:(c + 1) * LSH] = (
            dev.transpose(0, 2, 5, 1, 4, 3, 6).reshape(B, LSH, L, PAIR)
            .astype(np.float32) + b_out)
    return full


if __name__ == "__main__":
    # self-test with NON-trivial gamma/beta/mask against a numpy reference
    rng = np.random.default_rng(1)
    mask = np.ones((B, L), dtype=bool)
    mask[0, 500:] = False        # exercise the mask path
    mask[1, :3] = False
    inputs = {
        "node": rng.standard_normal((B, L, D)).astype(np.float32),
        "mask": mask,
        "ln_gamma": (1.0 + 0.1 * rng.standard_normal(D)).astype(np.float32),
        "ln_beta": (0.1 * rng.standard_normal(D)).astype(np.float32),
        "W_left": (rng.standard_normal((D, DH)) / np.sqrt(D)).astype(np.float32),
        "b_left": (0.1 * rng.standard_normal(DH)).astype(np.float32),
        "W_right": (rng.standard_normal((D, DH)) / np.sqrt(D)).astype(np.float32),
        "b_right": (0.1 * rng.standard_normal(DH)).astype(np.float32),
        "W_out": (rng.standard_normal((H, PAIR)) / np.sqrt(H)).astype(np.float32),
        "b_out": (0.1 * rng.standard_normal(PAIR)).astype(np.float32),
    }

    def np_reference(node, mask, ln_gamma, ln_beta, W_left, b_left, W_right,
                     b_right, W_out, b_out):
        node = node.astype(np.float64)
        mu = node.mean(-1, keepdims=True)
        var = ((node - mu) ** 2).mean(-1, keepdims=True)
        x = (node - mu) / np.sqrt(var + LN_EPS) * ln_gamma + ln_beta
        x = x * mask[..., None]
        left = (x @ W_left + b_left).reshape(B, L, H, -1)
        right = ((x @ W_right + b_right) / np.sqrt(DH)).reshape(B, L, H, -1)
        o = np.einsum("bihk,bjhk->bijh", left, right)
        return np.einsum("bijh,hp->bijp", o, W_out) + b_out

    got = kernel(**inputs)
    exp = np_reference(**inputs)
    rel = np.abs(got - exp).max() / np.abs(exp).max()
    print("general-path rel err:", rel)
    assert rel < 5e-3, rel
    print("OK", got.shape, got.dtype)


# revision 54
# speedup vs baseline: 1.0889x; 1.0064x over previous
"""Trainium2 Bass kernel for nn_Node2Pair_bias (LayerNorm -> dual projection ->
pair outer-product -> head-mix linear).

Reference computation (B=2, L=512, D=256, DH=32, H=16, K=2, P=128):
    x   = LayerNorm(node) * gamma + beta, masked        [B, L, D]
    left  = (x @ W_left + b_left)                       [B, L, DH] -> [B,L,H,K]
    right = (x @ W_right + b_right)/sqrt(DH)            [B, L, DH] -> [B,L,H,K]
    out[b,i,j,h] = sum_k left[b,i,h,k]*right[b,j,h,k]
    out[b,i,j,p] = sum_h out[b,i,j,h]*W_out[h,p] + b_out[p]   [B, L, L, P]

Mathematical restructuring (c = (h,k) combined channel, 0..31):
    out[b,i,j,p] = sum_c right[b,j,c] * (left[b,i,c] * W2[c,p]) + b_out[p]
with W2[c,p] = W_out[c//2, p].  For each i, M_i[c,p] = left[b,i,c]*W2[c,p] is
built on a vector-class engine; 4 i's pack side by side into an rhs of
[32, 512], and the K=32 contraction uses only one 32-row group of the PE
array — so 4 consecutive i-blocks (il=0..3) are row-packed via
tile_position=(32*il, 0) and run CONCURRENTLY on disjoint row groups:
  lhsT = rightT_quad[32il:32il+32, j-chunk]   (right values, 4 replicas)
  rhs  = mp_quad[32il:32il+32, (i4, p)=512]
  -> psum_il[j=128, (i4, p)=512]
The partition-replication of rightT/leftT across the 4 row groups comes free
by tiling the projection-weight COLUMNS 4x on the host.  PSUM is drained to
fp16 staging (DVE/ACT) and DMA'd out; the host adds b_out and converts
fp16 -> f32 while un-sharding (the 2e-2 rel-err budget is ~40x the fp16
rounding error).

LayerNorm gamma/beta and both projection biases are folded into the
projection weights (exact algebra): rows = [gamma[:,None]*W; (beta@W) paired
with a mask row; b paired with a ones row].

Sharding: the i axis of L is split across the 8 cores (sequence-parallel);
each core holds its [B, 64] slice of `left` inputs plus the full `right` side
and writes a [B, 64, L, P] output shard.  No cross-device communication.
"""

import sys

sys.path.insert(0, "/opt/trn_rl_repo")

import numpy as np

import concourse.bass as bass  # noqa: F401
import concourse.mybir as mybir
import concourse.tile as tile
from concourse import bacc
from concourse.bass_utils import run_bass_kernel_spmd
from concourse.masks import make_identity

F32 = mybir.dt.float32
F16 = mybir.dt.float16

B, L, D = 2, 512, 256
DH, H, PAIR = 32, 16, 128
NCORES = 8
LSH = L // NCORES          # 64 i's per core per batch
LN_EPS = 1e-5

_COMPILED = None  # (nc, input_names)


def _build_program():
    nc = bacc.Bacc("TRN2", target_bir_lowering=False, debug=False,
                   num_devices=NCORES)

    # ---------------- DRAM parameters ----------------
    def din(name, shape, dt=F32):
        return nc.dram_tensor(name, list(shape), dt, kind="ExternalInput").ap()

    node_full = din("node_full", (B * L, D))        # all rows, (b,l) major
    node_shard = din("node_shard", (B * LSH, D))    # this core's i rows, (b,i)
    mask_col_full = din("mask_col_full", (128, B * L // 128))  # [:, t] = tile t
    mask_col_shard = din("mask_col_shard", (128, 1))
    m2_full = din("m2_full", (B * 2, L), F16)       # per b: [mask row; ones]
    m2_shard = din("m2_shard", (2, B * LSH), F16)   # [mask row; ones]
    # columns tiled 4x (col 32*r + dh = W[:, dh]) so projections emit the
    # 4-replica partition layout row-packing needs
    w_left_e = din("w_left_e", (D + 2, 4 * DH), F16)   # [gamma*W; beta@W; b_l]
    w_right_e = din("w_right_e", (D + 2, 4 * DH), F16)  # scaled by 1/sqrt(DH)
    w2 = din("w2", (4 * DH, 4 * PAIR), F16)  # quad rows, free dim tiled 4x

    # Output layout: [b, jcp, sg, j, jh, i16, p] fp16 — each 1 MiB staging
    # buffer lands as one fully contiguous partition-major stream (8 KiB per
    # partition).  Host un-permutes and upcasts while assembling the output.
    out = nc.dram_tensor("out", [B, 2, 4, 128, 2, 16, PAIR], F16,
                         kind="ExternalOutput").ap()

    NT_FULL = B * L // 128   # 8 LayerNorm tiles for the full sequence

    with tile.TileContext(nc) as tc:
        with (
            tc.tile_pool(name="singles", bufs=1) as singles,
            tc.tile_pool(name="xpool", bufs=9) as xpool,
            tc.tile_pool(name="stats", bufs=4) as stats,
            tc.tile_pool(name="persist", bufs=1) as persist,
            tc.tile_pool(name="mp", bufs=4) as mp_pool,
            tc.tile_pool(name="stag", bufs=6) as stag_pool,
            tc.tile_pool(name="ps_tp", bufs=1, space="PSUM") as ps_tp,
            tc.tile_pool(name="ps_proj", bufs=1, space="PSUM") as ps_proj,
            tc.tile_pool(name="ps_big", bufs=3, space="PSUM") as ps_big,
        ):
            # ---------------- constants ----------------
            ident = singles.tile([128, 128], F32, tag="ident")
            make_identity(nc, ident)
            eps_t = singles.tile([128, 1], F32, tag="eps")
            nc.vector.memset(eps_t, LN_EPS)

            # hot-path loads on sync (HWDGE) in dependency-critical order
            # (small weights first so nothing queues behind the node tiles);
            # b=1 tiles + right-side constants via gpsimd (SWDGE) in parallel
            xs = xpool.tile([128, D], F32, tag="x", name="xs")
            nc.sync.dma_start(out=xs, in_=node_shard[:, :])
            mcs_sb = singles.tile([128, 1], F32, tag="mcs")
            nc.sync.dma_start(out=mcs_sb, in_=mask_col_shard[:, :])
            wl_sb = [singles.tile([128, 4 * DH], F16, tag=f"wl{dc}",
                                  name=f"wl{dc}") for dc in range(2)]
            for dc in range(2):
                nc.sync.dma_start(out=wl_sb[dc],
                                  in_=w_left_e[dc * 128:(dc + 1) * 128, :])
            wl_mo = singles.tile([2, 4 * DH], F16, tag="wlmo")
            nc.sync.dma_start(out=wl_mo, in_=w_left_e[D:D + 2, :])
            w2_sb = singles.tile([4 * DH, 4 * PAIR], F16, tag="w2")
            nc.sync.dma_start(out=w2_sb, in_=w2[:, :])
            # shard mask+ones rhs rows [2, B*LSH]
            m2s = singles.tile([2, B * LSH], F16, tag="m2s")
            nc.sync.dma_start(out=m2s, in_=m2_shard[:, :])
            # b=0 node tiles + mask on the scalar HWDGE queue — the ACT
            # sequencer is idle this early, so these land in parallel with
            # the sync queue's weight loads instead of queuing behind them
            mcf_sb = singles.tile([128, NT_FULL], F32, tag="mcf")
            nc.scalar.dma_start(out=mcf_sb, in_=mask_col_full[:, :])
            xf_tiles = [None] * NT_FULL
            for t in range(4):
                xf = xpool.tile([128, D], F32, tag="x", name=f"xf{t}")
                nc.scalar.dma_start(out=xf,
                                    in_=node_full[t * 128:(t + 1) * 128, :])
                xf_tiles[t] = xf

            wr_sb = [singles.tile([128, 4 * DH], F16, tag=f"wr{dc}",
                                  name=f"wr{dc}") for dc in range(2)]
            for dc in range(2):
                nc.gpsimd.dma_start(out=wr_sb[dc],
                                    in_=w_right_e[dc * 128:(dc + 1) * 128, :])
            wr_mo = singles.tile([2, 4 * DH], F16, tag="wrmo")
            nc.gpsimd.dma_start(out=wr_mo, in_=w_right_e[D:D + 2, :])
            m2f = [singles.tile([2, L], F16, tag=f"m2f{b}", name=f"m2f{b}")
                   for b in range(B)]
            for b in range(B):
                nc.gpsimd.dma_start(out=m2f[b],
                                    in_=m2_full[2 * b:2 * b + 2, :])
            for t in range(4, NT_FULL):
                xf = xpool.tile([128, D], F32, tag="x", name=f"xf{t}")
                nc.gpsimd.dma_start(out=xf,
                                    in_=node_full[t * 128:(t + 1) * 128, :])
                xf_tiles[t] = xf

            # ---------------- LayerNorm helper ----------------
            def layernorm_masked(x_t, mask_col_ap):
                """x_t [128, D] in place -> (x - mu) * rsqrt(var+eps) * mask."""
                st = stats.tile([128, 6], F32, tag="st")
                nc.vector.bn_stats(out=st, in_=x_t)
                mv = stats.tile([128, 2], F32, tag="mv")
                nc.vector.bn_aggr(out=mv, in_=st)
                sd = stats.tile([128, 1], F32, tag="sd")
                nc.scalar.activation(out=sd, in_=mv[:, 1:2],
                                     func=mybir.ActivationFunctionType.Sqrt,
                                     bias=eps_t, scale=1.0)
                rs = stats.tile([128, 1], F32, tag="rs")
                nc.vector.reciprocal(out=rs, in_=sd)
                rsm = stats.tile([128, 1], F32, tag="rsm")
                nc.vector.tensor_mul(out=rsm, in0=rs, in1=mask_col_ap)
                nc.vector.tensor_scalar(out=x_t, in0=x_t,
                                        scalar1=mv[:, 0:1], scalar2=rsm,
                                        op0=mybir.AluOpType.subtract,
                                        op1=mybir.AluOpType.mult)

            # ---------------- shard path: leftT_r [128, B*LSH] ---------------
            layernorm_masked(xs, mcs_sb[:, 0:1])

            xsT = [persist.tile([128, B * LSH], F16, tag=f"xsT{dc}",
                                name=f"xsT{dc}") for dc in range(2)]
            for dc in range(2):
                pt = ps_tp.tile([128, 128], F32, tag="tp")
                nc.tensor.transpose(pt, xs[:, dc * 128:(dc + 1) * 128], ident)
                nc.scalar.copy(out=xsT[dc], in_=pt)

            ps_l = ps_proj.tile([128, L], F32, tag="pr", name="ps_l")
            ps_l = ps_l[:, 0:B * LSH]
            for dc in range(2):
                nc.tensor.matmul(ps_l, wl_sb[dc], xsT[dc],
                                 start=(dc == 0), stop=False)
            nc.tensor.matmul(ps_l, wl_mo, m2s, start=False, stop=True)
            # leftT_r: per il row-group, columns permuted to (b, sg, q) so the
            # M_pack build's in1 column index is independent of the row group:
            # leftT_r[32il+c, b*16+sg*4+q] = left[b*64+sg*16+il*4+q, c]
            leftT = persist.tile([128, 32], F16, tag="leftT")
            for il in range(4):
                psl = slice(32 * il, 32 * il + 32)
                src = bass.AP(ps_l.tensor, ps_l[psl, il * 4:].offset,
                              [list(ps_l[psl, :].ap[0]),
                               [64, B], [16, 4], [1, 4]])
                dst = leftT[psl, :].rearrange("c (b s q) -> c b s q", b=B, q=4)
                nc.vector.tensor_copy(out=dst, in_=src)

            # ---------------- full path helper: rightT[b] [128, L] -----------
            xT = [[persist.tile([128, L], F16, tag=f"xT{b}_{dc}",
                                name=f"xT{b}_{dc}")
                   for dc in range(2)] for b in range(B)]
            rightT = [persist.tile([128, L], F16, tag=f"rt{b}",
                                   name=f"rt{b}") for b in range(B)]

            def full_path_ln(b, lc):
                t = b * 4 + lc
                xf = xf_tiles[t]
                layernorm_masked(xf, mcf_sb[:, t:t + 1])
                for dc in range(2):
                    pt = ps_tp.tile([128, 128], F32, tag="tp")
                    nc.tensor.transpose(pt, xf[:, dc * 128:(dc + 1) * 128],
                                        ident)
                    nc.scalar.copy(out=xT[b][dc][:, lc * 128:(lc + 1) * 128],
                                   in_=pt)

            def full_path_proj(b):
                ps_r = ps_proj.tile([128, L], F32, tag="pr")
                for jc in range(4):
                    jsl = slice(jc * 128, (jc + 1) * 128)
                    for dc in range(2):
                        nc.tensor.matmul(ps_r[:, jsl], wr_sb[dc],
                                         xT[b][dc][:, jsl],
                                         start=(dc == 0), stop=False)
                    nc.tensor.matmul(ps_r[:, jsl], wr_mo, m2f[b][:, jsl],
                                     start=False, stop=True)
                nc.vector.tensor_copy(out=rightT[b], in_=ps_r)

            # ---------------- M_pack builds ----------------
            # One DVE op per (b, sg): mp[32il+c, q*128+p] =
            # leftT_r[32il+c, b*16+sg*4+q] * w2[32il+c, p] via a stride-0
            # broadcast AP on the q/p free dims.
            def build_mps(b, sg):
                mp = mp_pool.tile([128, 512], F16, tag="mp",
                                  name=f"mp{b}_{sg}")
                lsl = leftT[:, b * 16 + sg * 4:]
                bc = bass.AP(lsl.tensor, lsl.offset,
                             [list(lsl.ap[0]), [1, 4], [0, 128]])
                nc.vector.tensor_tensor(
                    out=mp[:, :].rearrange("c (q x) -> c q x", x=128),
                    in0=w2_sb[:, :].rearrange("c (q x) -> c q x", x=128),
                    in1=bc, op=mybir.AluOpType.mult)
                return mp

            # ---------------- main pair loop ----------------
            COPY_PAT = "svsvsvsvsvsvsvss"   # ACT 9 : DVE 7
            copy_cnt = [0]

            def main_loop(b, sg, extra=None, fine_dma=False):
                mp = build_mps(b, sg)
                for jcp in range(2):
                    stg = stag_pool.tile([128, 4096], F16, tag="stag")
                    for jh in range(2):
                        jc = jcp * 2 + jh
                        jsl = slice(jc * 128, (jc + 1) * 128)
                        pbs = [ps_big.tile([128, 1024], F32, tag="big",
                                           name=f"pb{h2}")
                               for h2 in range(2)]
                        for il in range(4):
                            psl = slice(32 * il, 32 * il + 32)
                            nc.tensor.matmul(
                                pbs[il // 2][:, (il % 2) * 512:
                                             (il % 2 + 1) * 512],
                                rightT[b][psl, jsl], mp[psl, :],
                                start=True, stop=True,
                                tile_position=(32 * il, 0))
                        for half in range(2):
                            dst = stg[:, jh * 2048 + half * 1024:
                                      jh * 2048 + (half + 1) * 1024]
                            if COPY_PAT[copy_cnt[0] % len(COPY_PAT)] == "s":
                                nc.scalar.copy(out=dst, in_=pbs[half])
                            else:
                                nc.vector.tensor_copy(out=dst, in_=pbs[half])
                            copy_cnt[0] += 1
                    if fine_dma:
                        for jh in range(2):
                            dst_ap = out[b, jcp, sg, :, jh, :, :]
                            src_ap = stg[:, jh * 2048:(jh + 1) * 2048]\
                                .rearrange("j (i p) -> j i p", p=128)
                            nc.sync.dma_start(out=dst_ap, in_=src_ap)
                    else:
                        dst_ap = out[b, jcp, sg, :, :, :, :]
                        src_ap = stg[:, :].rearrange(
                            "j (jh i p) -> j jh i p", jh=2, p=128)
                        nc.sync.dma_start(out=dst_ap, in_=src_ap)
                if extra is not None:
                    extra()

            # b=0 full path, then its main loop; b=1's LayerNorm/transpose/
            # projection work is interleaved between b=0's sg groups so the
            # PE and DVE never sit idle waiting for b=1 inputs.
            for lc in range(4):
                full_path_ln(0, lc)
            full_path_proj(0)

            b1_stages = [
                lambda: full_path_ln(1, 0),
                lambda: full_path_ln(1, 1),
                lambda: full_path_ln(1, 2),
                lambda: (full_path_ln(1, 3), full_path_proj(1)),
            ]
            for sg in range(4):
                main_loop(0, sg, extra=b1_stages[sg], fine_dma=(sg == 0))
            for sg in range(4):
                main_loop(1, sg, fine_dma=(sg == 3))

    nc.compile()
    names = ["node_full", "node_shard", "mask_col_full", "mask_col_shard",
             "m2_full", "m2_shard", "w_left_e", "w_right_e", "w2"]
    return nc, names


def _prepare_in_maps(node, mask, ln_gamma, ln_beta, W_left, b_left, W_right,
                     b_right, W_out, b_out):
    f = np.float32
    node = np.ascontiguousarray(np.asarray(node, dtype=f))        # [B, L, D]
    mask_f = np.asarray(mask).astype(f)                           # [B, L]
    gamma = np.asarray(ln_gamma, dtype=f)
    beta = np.asarray(ln_beta, dtype=f)
    W_l = np.asarray(W_left, dtype=f)
    W_r = np.asarray(W_right, dtype=f)
    b_l = np.asarray(b_left, dtype=f)
    b_r = np.asarray(b_right, dtype=f)
    W_o = np.asarray(W_out, dtype=f)
    b_o = np.asarray(b_out, dtype=f)

    s = 1.0 / np.sqrt(np.float32(DH))
    w_left_e = np.tile(np.concatenate(
        [gamma[:, None] * W_l, (beta @ W_l)[None, :], b_l[None, :]], 0),
        (1, 4))
    w_right_e = np.tile(np.concatenate(
        [gamma[:, None] * W_r, (beta @ W_r)[None, :], b_r[None, :]], 0),
        (1, 4)) * s
    w2 = np.tile(np.tile(np.repeat(W_o, 2, axis=0), (4, 1)), (1, 4))

    node_flat = node.reshape(B * L, D)
    mask_col_full = np.ascontiguousarray(mask_f.reshape(-1, 128).T)  # [128, 8]

    f16 = np.float16
    m2_full = np.empty((B * 2, L), dtype=f16)
    for b in range(B):
        m2_full[2 * b] = mask_f[b].astype(f16)
        m2_full[2 * b + 1] = 1.0
    common = {
        "node_full": node_flat,
        "mask_col_full": mask_col_full,
        "m2_full": m2_full,
        "w_left_e": np.ascontiguousarray(w_left_e.astype(f16)),
        "w_right_e": np.ascontiguousarray(w_right_e.astype(f16)),
        "w2": np.ascontiguousarray(w2.astype(f16)),
    }

    in_maps = []
    for c in range(NCORES):
        sl = slice(c * LSH, (c + 1) * LSH)
        shard = np.ascontiguousarray(node[:, sl, :].reshape(B * LSH, D))
        msk = mask_f[:, sl]                                       # [B, LSH]
        m = dict(common)
        m["node_shard"] = shard
        m["mask_col_shard"] = np.ascontiguousarray(msk.reshape(-1)[:, None])
        m2_sh = np.empty((2, B * LSH), dtype=f16)
        m2_sh[0] = msk.reshape(-1).astype(f16)
        m2_sh[1] = 1.0
        m["m2_shard"] = m2_sh
        in_maps.append(m)
    return in_maps


def kernel(**inputs):
    global _COMPILED
    if _COMPILED is None:
        _COMPILED = _build_program()
    nc, names = _COMPILED
    in_maps = _prepare_in_maps(**inputs)
    res = run_bass_kernel_spmd(nc, in_maps, core_ids=list(range(NCORES)))
    b_out = np.asarray(inputs["b_out"], dtype=np.float32)
    full = np.empty((B, L, L, PAIR), np.float32)
    for c in range(NCORES):
        dev = res.results[c]["out"]   # [b, jcp, sg, j, jh, i16, p] fp16
        full[:, c * LSH:(c + 1) * LSH] = (
            dev.transpose(0, 2, 5, 1, 4, 3, 6).reshape(B, LSH, L, PAIR)
            .astype(np.float32) + b_out)
    return full


if __name__ == "__main__":
    # self-test with NON-trivial gamma/beta/mask against a numpy reference
    rng = np.random.default_rng(1)
    mask = np.ones((B, L), dtype=bool)
    mask[0, 500:] = False        # exercise the mask path
    mask[1, :3] = False
    inputs = {
        "node": rng.standard_normal((B, L, D)).astype(np.float32),
        "mask": mask,
        "ln_gamma": (1.0 + 0.1 * rng.standard_normal(D)).astype(np.float32),
        "ln_beta": (0.1 * rng.standard_normal(D)).astype(np.float32),
        "W_left": (rng.standard_normal((D, DH)) / np.sqrt(D)).astype(np.float32),
        "b_left": (0.1 * rng.standard_normal(DH)).astype(np.float32),
        "W_right": (rng.standard_normal((D, DH)) / np.sqrt(D)).astype(np.float32),
        "b_right": (0.1 * rng.standard_normal(DH)).astype(np.float32),
        "W_out": (rng.standard_normal((H, PAIR)) / np.sqrt(H)).astype(np.float32),
        "b_out": (0.1 * rng.standard_normal(PAIR)).astype(np.float32),
    }

    def np_reference(node, mask, ln_gamma, ln_beta, W_left, b_left, W_right,
                     b_right, W_out, b_out):
        node = node.astype(np.float64)
        mu = node.mean(-1, keepdims=True)
        var = ((node - mu) ** 2).mean(-1, keepdims=True)
        x = (node - mu) / np.sqrt(var + LN_EPS) * ln_gamma + ln_beta
        x = x * mask[..., None]
        left = (x @ W_left + b_left).reshape(B, L, H, -1)
        right = ((x @ W_right + b_right) / np.sqrt(DH)).reshape(B, L, H, -1)
        o = np.einsum("bihk,bjhk->bijh", left, right)
        return np.einsum("bijh,hp->bijp", o, W_out) + b_out

    got = kernel(**inputs)
    exp = np_reference(**inputs)
    rel = np.abs(got - exp).max() / np.abs(exp).max()
    print("general-path rel err:", rel)
    assert rel < 5e-3, rel
    print("OK", got.shape, got.dtype)
